# revision 36
# baseline (speedup 1.0000x reference)
"""MiniSTU Trainium2 kernel.

Reformulation (no FFT): the reference computes, per batch b,
    out = sum_k T_k @ (x @ Mp_k)  +  sgn ⊙ sum_k T_k @ (sgn ⊙ (x @ Mm_k))
where T_k is the lower-triangular Toeplitz matrix T_k[l,s] = phi[l-s,k]
and sgn[l] = (-1)^l.  (The sign-flipped filter Toeplitz satisfies
T~ = S T S with S = diag(sgn), which lets both branches share one set of
Toeplitz blocks.)

On device (per core; 8 cores = batch(2) x output-quarter(4), no collectives):
  stage 1:  A[l, (k,s,o)] = x_b^T-chunks @ M-matrix   (contraction D=512)
            with the minus branch sign-fixed on the PSUM->SBUF copy.
  stage 2:  out[I] = sum_{J<=I, k} PH[I-J,k]^T @ A[J, k-slice]
            (128x128 Toeplitz blocks of phi as stationary operands,
             free dim 256 = o-slice(128) x sign(2)).
Run twice over k-halves (8 filters each) to fit SBUF; fp16 operands,
fp32 PSUM accumulation.
"""

import numpy as np

B, L, D, O, K, P = 2, 2048, 512, 512, 16, 128
NB = L // P       # 16 l-blocks
KH = 2            # k halves
KPH = K // KH     # 8 filters per half
NOQ = 4           # o-quarters
OS = O // NOQ     # 128 per-core o slice
N_CORES = 8

_cache = {}


def _build_bass(reps=1, stages=(1, 2), mode="static"):
    import contextlib
    import concourse.mybir as mybir
    import concourse.tile as tile
    from concourse import bacc

    dt = mybir.dt
    f16, f32 = dt.float16, dt.float32

    nc = bacc.Bacc("TRN2", target_bir_lowering=False, debug=False,
                   num_devices=N_CORES)

    # DRAM layouts exactly match the SBUF tiles (host does all permutes).
    xt_d = nc.dram_tensor("xt", [P, 4, L], f16, kind="ExternalInput")
    mx_d = nc.dram_tensor("mx", [P, 4, K * 2 * OS], f16, kind="ExternalInput")
    ph_d = nc.dram_tensor("ph", [KH, 4, P, 4 * KPH * P], f16, kind="ExternalInput")
    sg_d = nc.dram_tensor("sg", [P, 1], f32, kind="ExternalInput")
    out_d = nc.dram_tensor("out", [P, NB * OS], f32, kind="ExternalOutput")

    CH = KPH * 2 * OS          # 2048 columns per k-half in mx/a
    with tile.TileContext(nc) as tc:
        with (
            tc.tile_pool(name="const", bufs=1) as cpool,
            tc.tile_pool(name="phpool", bufs=1) as phpool,
            tc.tile_pool(name="apool", bufs=1) as apool,
            tc.tile_pool(name="opool", bufs=1) as opool,
            contextlib.ExitStack() as psctx,
        ):
            if mode == "static":
                ps1pool = psctx.enter_context(
                    tc.tile_pool(name="ps1", bufs=2, space="PSUM"))
                ps2pool = psctx.enter_context(
                    tc.tile_pool(name="ps2", bufs=1, space="PSUM"))
            else:
                ps1pool = ps2pool = None
            xt = cpool.tile([P, 4, L], f16, tag="xt")
            mx = cpool.tile([P, 4, K * 2 * OS], f16, tag="mx")
            sg = cpool.tile([P, 1], f32, tag="sg")
            a_sb = apool.tile([P, NB, CH], f16, tag="a")
            outacc = opool.tile([P, NB, 2 * OS], f32, tag="outacc")
            res = opool.tile([P, NB, OS], f32, tag="res")

            for dc in range(4):
                nc.sync.dma_start(out=xt[:, dc, :], in_=xt_d[:, dc, :])
                nc.sync.dma_start(out=mx[:, dc, :], in_=mx_d[:, dc, :])
            nc.sync.dma_start(out=sg[:], in_=sg_d[:])
            if 1 not in stages or "no_s1_copy" in stages:
                nc.vector.memset(a_sb[:], 0.0)
            if 2 not in stages:
                nc.vector.memset(outacc[:], 0.0)

            loop_cm = tc.For_i(0, reps, 1) if reps > 1 else contextlib.nullcontext()
            with loop_cm:
                _emit_body(nc, tc, mybir, f16, f32, xt, mx, sg, ph_d, phpool,
                           a_sb, outacc, res, out_d, stages, ps1pool, ps2pool,
                           mode)

    nc.compile()
    return nc


def _emit_body(nc, tc, mybir, f16, f32, xt, mx, sg, ph_d, phpool,
               a_sb, outacc, res, out_d, stages=(1, 2), ps1pool=None,
               ps2pool=None, mode="static"):
    CH = KPH * 2 * OS
    no_s1_copy = "no_s1_copy" in stages
    # static: ps1 holds banks 0-3 / ps2 banks 4-7 for the whole kernel
    #   (no phase serialization; stage-2 runs four 4-accumulator passes).
    # swap: per-phase pools use all 8 banks (stage-2 runs two
    #   8-accumulator passes; phases serialize on PSUM reuse).
    s2_grp = 4 if mode == "static" else 8

    import contextlib as _ctx
    for kh in range(KH):
        ph = phpool.tile([P, NB * KPH * P], f16, tag="ph")
        for q in range(4):
            nc.sync.dma_start(out=ph[:, q * 4096:(q + 1) * 4096],
                              in_=ph_d[kh, q])

        # ---- stage 1: A[lt] = sum_dc xt[dc,lt]^T.T @ mx[dc, half]
        # Each 512-col psum region is one full bank (interleaved groups
        # are safe only at bank granularity: start=True clears
        # whole-bank has_written bits).
        s1cm = (tc.tile_pool(name="ps1", bufs=2, space="PSUM")
                if mode == "swap" else _ctx.nullcontext(ps1pool))
        with s1cm as pool1:
            nt = 1 if mode == "swap" else 2      # psum tiles per lt
            ncols = 2048 // nt
            for lt in range(NB) if 1 in stages else []:
                for t in range(nt):
                    ps = pool1.tile([P, ncols], f32, tag="ps1")
                    for dc in range(4):
                        for n in range(ncols // 512):
                            c0 = kh * CH + t * ncols + n * 512
                            nc.tensor.matmul(
                                ps[:, n * 512:(n + 1) * 512],
                                xt[:, dc, lt * P:(lt + 1) * P],
                                mx[:, dc, c0:c0 + 512],
                                start=(dc == 0), stop=(dc == 3),
                            )
                    # copy to A (fp16), applying sgn to the minus columns
                    if not no_s1_copy:
                        na = ncols // 256
                        psv = ps[:].rearrange("p (a s o) -> p a s o",
                                              a=na, s=2, o=OS)
                        av = a_sb[:, lt, t * ncols:(t + 1) * ncols].rearrange(
                            "p (a s o) -> p a s o", a=na, s=2, o=OS)
                        nc.vector.tensor_copy(av[:, :, 0, :], psv[:, :, 0, :])
                        nc.vector.tensor_scalar_mul(
                            av[:, :, 1, :], psv[:, :, 1, :], sg[:])

        # ---- stage 2: out[I] += sum_{J<=I,k} PH[I-J,k].T @ A[J,k]
        # (d,kl)-outer weight reuse; one accumulator per PSUM bank.
        s2cm = (tc.tile_pool(name="ps2", bufs=1, space="PSUM")
                if mode == "swap" else _ctx.nullcontext(ps2pool))
        with s2cm as pool2:
            for ig in range(NB // s2_grp) if 2 in stages else []:
                i_lo = ig * s2_grp
                ps2 = pool2.tile([P, s2_grp, 512], f32, tag="ps2")
                for d in range(NB):
                    j_lo = max(0, i_lo - d)
                    j_hi = min(NB, i_lo + s2_grp - d)
                    if j_hi <= j_lo:
                        continue
                    for kl in range(KPH):
                        for J in range(j_lo, j_hi):
                            I = J + d
                            nc.tensor.matmul(
                                ps2[:, I - i_lo, 0:2 * OS],
                                ph[:, (d * KPH + kl) * P:(d * KPH + kl + 1) * P],
                                a_sb[:, J, kl * 2 * OS:(kl + 1) * 2 * OS],
                                start=(d == 0 and kl == 0),
                                stop=(d == I and kl == KPH - 1),
                            )
                    # accumulator I == d closes after its (d == I) pass
                    if i_lo <= d < i_lo + s2_grp:
                        I = d
                        if kh == 0:
                            nc.vector.tensor_copy(
                                outacc[:, I, :], ps2[:, I - i_lo, 0:2 * OS])
                        else:
                            nc.vector.tensor_add(
                                outacc[:, I, :], outacc[:, I, :],
                                ps2[:, I - i_lo, 0:2 * OS])

    # ---- final: res = plus + sgn * minus
    ov = outacc[:].rearrange("p i (s o) -> p i s o", s=2, o=OS)
    nc.vector.scalar_tensor_tensor(
        res[:], ov[:, :, 1, :], sg[:], ov[:, :, 0, :],
        op0=mybir.AluOpType.mult, op1=mybir.AluOpType.add,
    )
    nc.sync.dma_start(out=out_d[:], in_=res[:].rearrange("p i o -> p (i o)"))


def _prep_inputs(x, phi, M_phi_plus, M_phi_minus):
    """Host-side shard prep. Returns list of 8 input dicts (cores = b*4 + oq)."""
    sgn = np.where(np.arange(L) % 2 == 1, -1.0, 1.0).astype(np.float32)

    # xt[p, dc, l] = x[b, l, dc*128+p]
    xts = []
    for b in range(B):
        xt = np.ascontiguousarray(
            x[b].T.reshape(4, P, L).transpose(1, 0, 2)).astype(np.float16)
        xts.append(xt)

    # mx[p, dc, k*256 + s*128 + oo] = M_s[k, dc*128+p, oq*128+oo]
    mcat = np.stack([M_phi_plus, M_phi_minus], axis=1)  # [K, 2, D, O]
    mxs = []
    for oq in range(NOQ):
        m = mcat[:, :, :, oq * OS:(oq + 1) * OS]        # [K, 2, D, OS]
        m = m.transpose(2, 0, 1, 3).reshape(D, K * 2 * OS)  # [D, K*2*OS]
        mx = np.ascontiguousarray(
            m.reshape(4, P, K * 2 * OS).transpose(1, 0, 2)).astype(np.float16)
        mxs.append(mx)

    # ph[kh, q, pp, ((dq, kl, p))] = phi[d*P + p - pp, kh*KPH+kl], d = 4q+dq
    idx = np.arange(P)
    diff = idx[None, :] - idx[:, None]                  # [pp, p] = p - pp
    v = np.arange(NB)[:, None, None] * P + diff[None]   # [d, pp, p]
    valid = v >= 0
    phb = np.zeros((NB, P, P, K), dtype=np.float32)     # [d, pp, p, k]
    phb[valid] = phi[v[valid], :]
    # [d, pp, p, (kh, kl)] -> [kh, q, pp, dq, kl, p]
    phb = phb.reshape(4, 4, P, P, KH, KPH).transpose(4, 0, 2, 1, 5, 3)
    ph = np.ascontiguousarray(phb.reshape(KH, 4, P, 4 * KPH * P)).astype(np.float16)

    sg = np.ascontiguousarray(sgn[:P].reshape(P, 1))

    in_maps = []
    for b in range(B):
        for oq in range(NOQ):
            in_maps.append({"xt": xts[b], "mx": mxs[oq], "ph": ph, "sg": sg})
    return in_maps


def kernel(x, phi, M_phi_plus, M_phi_minus):
    from concourse.bass_utils import run_bass_kernel_spmd

    x = np.asarray(x, dtype=np.float32)
    phi = np.asarray(phi, dtype=np.float32)
    M_phi_plus = np.asarray(M_phi_plus, dtype=np.float32)
    M_phi_minus = np.asarray(M_phi_minus, dtype=np.float32)

    if "nc" not in _cache:
        _cache["nc"] = _build_bass()
    nc = _cache["nc"]

    in_maps = _prep_inputs(x, phi, M_phi_plus, M_phi_minus)
    results = run_bass_kernel_spmd(nc, in_maps, core_ids=list(range(N_CORES)))

    out = np.empty((B, L, O), dtype=np.float32)
    for c in range(N_CORES):
        b, oq = divmod(c, NOQ)
        r = results.results[c]["out"]                   # [P, NB*OS]
        blk = r.reshape(P, NB, OS).transpose(1, 0, 2).reshape(L, OS)
        out[b, :, oq * OS:(oq + 1) * OS] = blk
    return out


# revision 37
# speedup vs baseline: 1.0364x; 1.0364x over previous
"""MiniSTU Trainium2 kernel.

Reformulation (no FFT): the reference computes, per batch b,
    out = sum_k T_k @ (x @ Mp_k)  +  sgn ⊙ sum_k T_k @ (sgn ⊙ (x @ Mm_k))
where T_k is the lower-triangular Toeplitz matrix T_k[l,s] = phi[l-s,k]
and sgn[l] = (-1)^l.  (The sign-flipped filter Toeplitz satisfies
T~ = S T S with S = diag(sgn), which lets both branches share one set of
Toeplitz blocks.)

On device (per core; 8 cores = batch(2) x output-quarter(4), no collectives):
  stage 1:  A[l, (k,s,o)] = x_b^T-chunks @ M-matrix   (contraction D=512)
            with the minus branch sign-fixed on the PSUM->SBUF copy.
  stage 2:  out[I] = sum_{J<=I, k} PH[I-J,k]^T @ A[J, k-slice]
            (128x128 Toeplitz blocks of phi as stationary operands,
             free dim 256 = o-slice(128) x sign(2)).
Run twice over k-halves (8 filters each) to fit SBUF; fp16 operands,
fp32 PSUM accumulation.
"""

import numpy as np

B, L, D, O, K, P = 2, 2048, 512, 512, 16, 128
NB = L // P       # 16 l-blocks
KH = 2            # k halves
KPH = K // KH     # 8 filters per half
NOQ = 4           # o-quarters
OS = O // NOQ     # 128 per-core o slice
N_CORES = 8

_cache = {}


def _build_bass(reps=1, stages=(1, 2), mode="static"):
    import contextlib
    import concourse.mybir as mybir
    import concourse.tile as tile
    from concourse import bacc

    dt = mybir.dt
    f16, f32 = dt.float16, dt.float32

    nc = bacc.Bacc("TRN2", target_bir_lowering=False, debug=False,
                   num_devices=N_CORES)

    # DRAM layouts exactly match the SBUF tiles (host does all permutes).
    xt_d = nc.dram_tensor("xt", [P, 4, L], f16, kind="ExternalInput")
    mx_d = nc.dram_tensor("mx", [P, 4, K * 2 * OS], f16, kind="ExternalInput")
    ph_d = nc.dram_tensor("ph", [KH, 4, P, 4 * KPH * P], f16, kind="ExternalInput")
    sg_d = nc.dram_tensor("sg", [P, 1], f32, kind="ExternalInput")
    out_d = nc.dram_tensor("out", [P, NB * OS], f32, kind="ExternalOutput")

    CH = KPH * 2 * OS          # 2048 columns per k-half in mx/a
    with tile.TileContext(nc) as tc:
        with (
            tc.tile_pool(name="const", bufs=1) as cpool,
            tc.tile_pool(name="phpool", bufs=1) as phpool,
            tc.tile_pool(name="apool", bufs=1) as apool,
            tc.tile_pool(name="opool", bufs=1) as opool,
            contextlib.ExitStack() as psctx,
        ):
            if mode == "static":
                ps1pool = psctx.enter_context(
                    tc.tile_pool(name="ps1", bufs=2, space="PSUM"))
                ps2pool = psctx.enter_context(
                    tc.tile_pool(name="ps2", bufs=1, space="PSUM"))
            else:
                ps1pool = ps2pool = None
            xt = cpool.tile([P, 4, L], f16, tag="xt")
            mx = cpool.tile([P, 4, K * 2 * OS], f16, tag="mx")
            sg = cpool.tile([P, 1], f32, tag="sg")
            a_sb = apool.tile([P, NB, CH], f16, tag="a")
            outacc = opool.tile([P, NB, 2 * OS], f32, tag="outacc")
            res = opool.tile([P, NB, OS], f32, tag="res")

            for dc in range(4):
                nc.sync.dma_start(out=xt[:, dc, :], in_=xt_d[:, dc, :])
                nc.sync.dma_start(out=mx[:, dc, :], in_=mx_d[:, dc, :])
            nc.sync.dma_start(out=sg[:], in_=sg_d[:])
            if 1 not in stages or "no_s1_copy" in stages:
                nc.vector.memset(a_sb[:], 0.0)
            if 2 not in stages:
                nc.vector.memset(outacc[:], 0.0)

            loop_cm = (tc.For_i(0, reps, 1,
                                hint_engines=(mybir.EngineType.PE,
                                              mybir.EngineType.DVE))
                       if reps > 1 else contextlib.nullcontext())
            with loop_cm:
                _emit_body(nc, tc, mybir, f16, f32, xt, mx, sg, ph_d, phpool,
                           a_sb, outacc, res, out_d, stages, ps1pool, ps2pool,
                           mode)

    nc.compile()
    return nc


def _emit_body(nc, tc, mybir, f16, f32, xt, mx, sg, ph_d, phpool,
               a_sb, outacc, res, out_d, stages=(1, 2), ps1pool=None,
               ps2pool=None, mode="static"):
    CH = KPH * 2 * OS
    no_s1_copy = "no_s1_copy" in stages
    # static: ps1 holds banks 0-3 / ps2 banks 4-7 for the whole kernel
    #   (no phase serialization; stage-2 runs four 4-accumulator passes).
    # swap: per-phase pools use all 8 banks (stage-2 runs two
    #   8-accumulator passes; phases serialize on PSUM reuse).
    s2_grp = 4 if mode == "static" else 8

    import contextlib as _ctx
    for kh in range(KH):
        ph = phpool.tile([P, NB * KPH * P], f16, tag="ph")
        for q in range(4):
            nc.sync.dma_start(out=ph[:, q * 4096:(q + 1) * 4096],
                              in_=ph_d[kh, q])

        # ---- stage 1: A[lt] = sum_dc xt[dc,lt]^T.T @ mx[dc, half]
        # Each 512-col psum region is one full bank (interleaved groups
        # are safe only at bank granularity: start=True clears
        # whole-bank has_written bits).
        s1cm = (tc.tile_pool(name="ps1", bufs=2, space="PSUM")
                if mode == "swap" else _ctx.nullcontext(ps1pool))
        with s1cm as pool1:
            nt = 1 if mode == "swap" else 2      # psum tiles per lt
            ncols = 2048 // nt
            for lt in range(NB) if 1 in stages else []:
                for t in range(nt):
                    ps = pool1.tile([P, ncols], f32, tag="ps1")
                    for dc in range(4):
                        for n in range(ncols // 512):
                            c0 = kh * CH + t * ncols + n * 512
                            nc.tensor.matmul(
                                ps[:, n * 512:(n + 1) * 512],
                                xt[:, dc, lt * P:(lt + 1) * P],
                                mx[:, dc, c0:c0 + 512],
                                start=(dc == 0), stop=(dc == 3),
                            )
                    # copy to A (fp16), applying sgn to the minus columns
                    if not no_s1_copy:
                        na = ncols // 256
                        psv = ps[:].rearrange("p (a s o) -> p a s o",
                                              a=na, s=2, o=OS)
                        av = a_sb[:, lt, t * ncols:(t + 1) * ncols].rearrange(
                            "p (a s o) -> p a s o", a=na, s=2, o=OS)
                        nc.vector.tensor_copy(av[:, :, 0, :], psv[:, :, 0, :])
                        nc.vector.tensor_scalar_mul(
                            av[:, :, 1, :], psv[:, :, 1, :], sg[:])

        # ---- stage 2: out[I] += sum_{J<=I,k} PH[I-J,k].T @ A[J,k]
        # (d,kl)-outer weight reuse; one accumulator per PSUM bank.
        s2cm = (tc.tile_pool(name="ps2", bufs=1, space="PSUM")
                if mode == "swap" else _ctx.nullcontext(ps2pool))
        with s2cm as pool2:
            for ig in range(NB // s2_grp) if 2 in stages else []:
                i_lo = ig * s2_grp
                ps2 = pool2.tile([P, s2_grp, 512], f32, tag="ps2")
                for d in range(NB):
                    j_lo = max(0, i_lo - d)
                    j_hi = min(NB, i_lo + s2_grp - d)
                    if j_hi <= j_lo:
                        continue
                    for kl in range(KPH):
                        for J in range(j_lo, j_hi):
                            I = J + d
                            nc.tensor.matmul(
                                ps2[:, I - i_lo, 0:2 * OS],
                                ph[:, (d * KPH + kl) * P:(d * KPH + kl + 1) * P],
                                a_sb[:, J, kl * 2 * OS:(kl + 1) * 2 * OS],
                                start=(d == 0 and kl == 0),
                                stop=(d == I and kl == KPH - 1),
                            )
                    # accumulator I == d closes after its (d == I) pass
                    if i_lo <= d < i_lo + s2_grp:
                        I = d
                        if kh == 0:
                            nc.vector.tensor_copy(
                                outacc[:, I, :], ps2[:, I - i_lo, 0:2 * OS])
                        else:
                            nc.vector.tensor_add(
                                outacc[:, I, :], outacc[:, I, :],
                                ps2[:, I - i_lo, 0:2 * OS])

    # ---- final: res = plus + sgn * minus
    ov = outacc[:].rearrange("p i (s o) -> p i s o", s=2, o=OS)
    nc.vector.scalar_tensor_tensor(
        res[:], ov[:, :, 1, :], sg[:], ov[:, :, 0, :],
        op0=mybir.AluOpType.mult, op1=mybir.AluOpType.add,
    )
    nc.sync.dma_start(out=out_d[:], in_=res[:].rearrange("p i o -> p (i o)"))


def _prep_inputs(x, phi, M_phi_plus, M_phi_minus):
    """Host-side shard prep. Returns list of 8 input dicts (cores = b*4 + oq)."""
    sgn = np.where(np.arange(L) % 2 == 1, -1.0, 1.0).astype(np.float32)

    # xt[p, dc, l] = x[b, l, dc*128+p]
    xts = []
    for b in range(B):
        xt = np.ascontiguousarray(
            x[b].T.reshape(4, P, L).transpose(1, 0, 2)).astype(np.float16)
        xts.append(xt)

    # mx[p, dc, k*256 + s*128 + oo] = M_s[k, dc*128+p, oq*128+oo]
    mcat = np.stack([M_phi_plus, M_phi_minus], axis=1)  # [K, 2, D, O]
    mxs = []
    for oq in range(NOQ):
        m = mcat[:, :, :, oq * OS:(oq + 1) * OS]        # [K, 2, D, OS]
        m = m.transpose(2, 0, 1, 3).reshape(D, K * 2 * OS)  # [D, K*2*OS]
        mx = np.ascontiguousarray(
            m.reshape(4, P, K * 2 * OS).transpose(1, 0, 2)).astype(np.float16)
        mxs.append(mx)

    # ph[kh, q, pp, ((dq, kl, p))] = phi[d*P + p - pp, kh*KPH+kl], d = 4q+dq
    idx = np.arange(P)
    diff = idx[None, :] - idx[:, None]                  # [pp, p] = p - pp
    v = np.arange(NB)[:, None, None] * P + diff[None]   # [d, pp, p]
    valid = v >= 0
    phb = np.zeros((NB, P, P, K), dtype=np.float32)     # [d, pp, p, k]
    phb[valid] = phi[v[valid], :]
    # [d, pp, p, (kh, kl)] -> [kh, q, pp, dq, kl, p]
    phb = phb.reshape(4, 4, P, P, KH, KPH).transpose(4, 0, 2, 1, 5, 3)
    ph = np.ascontiguousarray(phb.reshape(KH, 4, P, 4 * KPH * P)).astype(np.float16)

    sg = np.ascontiguousarray(sgn[:P].reshape(P, 1))

    in_maps = []
    for b in range(B):
        for oq in range(NOQ):
            in_maps.append({"xt": xts[b], "mx": mxs[oq], "ph": ph, "sg": sg})
    return in_maps


def kernel(x, phi, M_phi_plus, M_phi_minus):
    from concourse.bass_utils import run_bass_kernel_spmd

    x = np.asarray(x, dtype=np.float32)
    phi = np.asarray(phi, dtype=np.float32)
    M_phi_plus = np.asarray(M_phi_plus, dtype=np.float32)
    M_phi_minus = np.asarray(M_phi_minus, dtype=np.float32)

    if "nc" not in _cache:
        _cache["nc"] = _build_bass()
    nc = _cache["nc"]

    in_maps = _prep_inputs(x, phi, M_phi_plus, M_phi_minus)
    results = run_bass_kernel_spmd(nc, in_maps, core_ids=list(range(N_CORES)))

    out = np.empty((B, L, O), dtype=np.float32)
    for c in range(N_CORES):
        b, oq = divmod(c, NOQ)
        r = results.results[c]["out"]                   # [P, NB*OS]
        blk = r.reshape(P, NB, OS).transpose(1, 0, 2).reshape(L, OS)
        out[b, :, oq * OS:(oq + 1) * OS] = blk
    return out


# revision 42
# speedup vs baseline: 1.1083x; 1.0694x over previous
"""MiniSTU Trainium2 kernel.

Reformulation (no FFT): the reference computes, per batch b,
    out = sum_k T_k @ (x @ Mp_k)  +  sgn ⊙ sum_k T_k @ (sgn ⊙ (x @ Mm_k))
where T_k is the lower-triangular Toeplitz matrix T_k[l,s] = phi[l-s,k]
and sgn[l] = (-1)^l.  (The sign-flipped filter Toeplitz satisfies
T~ = S T S with S = diag(sgn), which lets both branches share one set of
Toeplitz blocks.)

On device (per core; 8 cores = batch(2) x output-quarter(4), no collectives):
  stage 1:  A[l, (k,s,o)] = x_b^T-chunks @ M-matrix   (contraction D=512)
            with the minus branch sign-fixed on the PSUM->SBUF copy.
  stage 2:  out[I] = sum_{J<=I, k} PH[I-J,k]^T @ A[J, k-slice]
            (128x128 Toeplitz blocks of phi as stationary operands,
             free dim 256 = o-slice(128) x sign(2)).
Run twice over k-halves (8 filters each) to fit SBUF; fp16 operands,
fp32 PSUM accumulation.
"""

import numpy as np

B, L, D, O, K, P = 2, 2048, 512, 512, 16, 128
NB = L // P       # 16 l-blocks
KH = 2            # k halves
KPH = K // KH     # 8 filters per half
NOQ = 4           # o-quarters
OS = O // NOQ     # 128 per-core o slice
N_CORES = 8

_cache = {}


def _build_bass(reps=1, stages=(1, 2), mode="static"):
    import contextlib
    import concourse.mybir as mybir
    import concourse.tile as tile
    from concourse import bacc

    dt = mybir.dt
    f16, f32 = dt.float16, dt.float32

    nc = bacc.Bacc("TRN2", target_bir_lowering=False, debug=False,
                   num_devices=N_CORES)

    # DRAM layouts exactly match the SBUF tiles (host does all permutes).
    xt_d = nc.dram_tensor("xt", [P, 4, L], f16, kind="ExternalInput")
    mx_d = nc.dram_tensor("mx", [P, 4, K * 2 * OS], f16, kind="ExternalInput")
    ph_d = nc.dram_tensor("ph", [KH, 4, P, 4 * KPH * P], f16, kind="ExternalInput")
    sg_d = nc.dram_tensor("sg", [P, 1], f32, kind="ExternalInput")
    out_d = nc.dram_tensor("out", [P, NB * OS], f32, kind="ExternalOutput")

    CH = KPH * 2 * OS          # 2048 columns per k-half in mx/a
    with tile.TileContext(nc) as tc:
        with (
            tc.tile_pool(name="const", bufs=1) as cpool,
            tc.tile_pool(name="phpool", bufs=1) as phpool,
            tc.tile_pool(name="apool", bufs=1) as apool,
            tc.tile_pool(name="opool", bufs=1) as opool,
            contextlib.ExitStack() as psctx,
        ):
            if mode == "static":
                ps1pool = psctx.enter_context(
                    tc.tile_pool(name="ps1", bufs=2, space="PSUM"))
                ps2pool = psctx.enter_context(
                    tc.tile_pool(name="ps2", bufs=1, space="PSUM"))
            else:
                ps1pool = ps2pool = None
            xt = cpool.tile([P, 4, L], f16, tag="xt")
            mx = cpool.tile([P, 4, K * 2 * OS], f16, tag="mx")
            sg = cpool.tile([P, 1], f32, tag="sg")
            a_sb = apool.tile([P, NB, CH], f16, tag="a")
            outacc = opool.tile([P, NB, 2 * OS], f32, tag="outacc")
            res = opool.tile([P, NB, OS], f32, tag="res")

            for dc in range(4):
                nc.sync.dma_start(out=xt[:, dc, :], in_=xt_d[:, dc, :])
                nc.sync.dma_start(out=mx[:, dc, :], in_=mx_d[:, dc, :])
            nc.sync.dma_start(out=sg[:], in_=sg_d[:])
            if 1 not in stages or "no_s1_copy" in stages:
                nc.vector.memset(a_sb[:], 0.0)
            if 2 not in stages:
                nc.vector.memset(outacc[:], 0.0)

            loop_cm = (tc.For_i(0, reps, 1,
                                hint_engines=(mybir.EngineType.PE,
                                              mybir.EngineType.DVE))
                       if reps > 1 else contextlib.nullcontext())
            with loop_cm:
                _emit_body(nc, tc, mybir, f16, f32, xt, mx, sg, ph_d, phpool,
                           a_sb, outacc, res, out_d, stages, ps1pool, ps2pool,
                           mode)

    nc.compile()
    return nc


def _emit_body(nc, tc, mybir, f16, f32, xt, mx, sg, ph_d, phpool,
               a_sb, outacc, res, out_d, stages=(1, 2), ps1pool=None,
               ps2pool=None, mode="static"):
    CH = KPH * 2 * OS
    no_s1_copy = "no_s1_copy" in stages
    # static: ps1 holds banks 0-3 / ps2 banks 4-7 for the whole kernel
    #   (no phase serialization; stage-2 runs four 4-accumulator passes).
    # swap: per-phase pools use all 8 banks (stage-2 runs two
    #   8-accumulator passes; phases serialize on PSUM reuse).
    s2_grp = 4 if mode == "static" else 8

    import contextlib as _ctx
    for kh in range(KH):
        ph = phpool.tile([P, NB * KPH * P], f16, tag="ph")
        for q in range(4):
            nc.sync.dma_start(out=ph[:, q * 4096:(q + 1) * 4096],
                              in_=ph_d[kh, q])

        # ---- stage 1: A[lt] = sum_dc xt[dc,lt]^T.T @ mx[dc, half]
        # Each 512-col psum region is one full bank (interleaved groups
        # are safe only at bank granularity: start=True clears
        # whole-bank has_written bits).
        s1cm = (tc.tile_pool(name="ps1", bufs=2, space="PSUM")
                if mode == "swap" else _ctx.nullcontext(ps1pool))
        with s1cm as pool1:
            nt = 1 if mode == "swap" else 2      # psum tiles per lt
            ncols = 2048 // nt
            for lt in range(NB) if 1 in stages else []:
                for t in range(nt):
                    ps = pool1.tile([P, ncols], f32, tag="ps1")
                    for dc in range(4):
                        for n in range(ncols // 512):
                            c0 = kh * CH + t * ncols + n * 512
                            nc.tensor.matmul(
                                ps[:, n * 512:(n + 1) * 512],
                                xt[:, dc, lt * P:(lt + 1) * P],
                                mx[:, dc, c0:c0 + 512],
                                start=(dc == 0), stop=(dc == 3),
                            )
                    # copy to A (fp16), applying sgn to the minus columns
                    if not no_s1_copy:
                        na = ncols // 256
                        psv = ps[:].rearrange("p (a s o) -> p a s o",
                                              a=na, s=2, o=OS)
                        av = a_sb[:, lt, t * ncols:(t + 1) * ncols].rearrange(
                            "p (a s o) -> p a s o", a=na, s=2, o=OS)
                        nc.vector.tensor_copy(av[:, :, 0, :], psv[:, :, 0, :])
                        nc.vector.tensor_scalar_mul(
                            av[:, :, 1, :], psv[:, :, 1, :], sg[:])

        # ---- stage 2: out[I] += sum_{J<=I,k} PH[I-J,k].T @ A[J,k]
        # (d,kl)-outer weight reuse; one accumulator per PSUM bank.
        s2cm = (tc.tile_pool(name="ps2", bufs=1, space="PSUM")
                if mode == "swap" else _ctx.nullcontext(ps2pool))
        with s2cm as pool2:
            for ig in range(NB // s2_grp) if 2 in stages else []:
                i_lo = ig * s2_grp
                ps2 = pool2.tile([P, s2_grp, 512], f32, tag="ps2")
                for d in range(NB):
                    j_lo = max(0, i_lo - d)
                    j_hi = min(NB, i_lo + s2_grp - d)
                    if j_hi <= j_lo:
                        continue
                    for kl in range(KPH):
                        for J in range(j_lo, j_hi):
                            I = J + d
                            nc.tensor.matmul(
                                ps2[:, I - i_lo, 0:2 * OS],
                                ph[:, (d * KPH + kl) * P:(d * KPH + kl + 1) * P],
                                a_sb[:, J, kl * 2 * OS:(kl + 1) * 2 * OS],
                                start=(d == 0 and kl == 0),
                                stop=(d == I and kl == KPH - 1),
                            )
                    # accumulator I == d closes after its (d == I) pass
                    if i_lo <= d < i_lo + s2_grp:
                        I = d
                        if kh == 0:
                            nc.vector.tensor_copy(
                                outacc[:, I, :], ps2[:, I - i_lo, 0:2 * OS])
                        else:
                            nc.vector.tensor_add(
                                outacc[:, I, :], outacc[:, I, :],
                                ps2[:, I - i_lo, 0:2 * OS])
                            # outacc[I] is final: combine signs and ship it
                            # now, overlapping the tail with remaining MMs
                            ovI = outacc[:, I, :].rearrange(
                                "p (s o) -> p s o", s=2, o=OS)
                            nc.vector.scalar_tensor_tensor(
                                res[:, I, :], ovI[:, 1, :], sg[:], ovI[:, 0, :],
                                op0=mybir.AluOpType.mult,
                                op1=mybir.AluOpType.add)
                            nc.sync.dma_start(
                                out=out_d[:, I * OS:(I + 1) * OS],
                                in_=res[:, I, :])

    # ---- final combine+DMA happen per-I inside the kh=1 drain; fall back
    # to a bulk pass only for stage-probe builds that skip stage 2.
    if 2 not in stages:
        ov = outacc[:].rearrange("p i (s o) -> p i s o", s=2, o=OS)
        nc.vector.scalar_tensor_tensor(
            res[:], ov[:, :, 1, :], sg[:], ov[:, :, 0, :],
            op0=mybir.AluOpType.mult, op1=mybir.AluOpType.add,
        )
        nc.sync.dma_start(out=out_d[:], in_=res[:].rearrange("p i o -> p (i o)"))


def _prep_inputs(x, phi, M_phi_plus, M_phi_minus):
    """Host-side shard prep. Returns list of 8 input dicts (cores = b*4 + oq)."""
    sgn = np.where(np.arange(L) % 2 == 1, -1.0, 1.0).astype(np.float32)

    # xt[p, dc, l] = x[b, l, dc*128+p]
    xts = []
    for b in range(B):
        xt = np.ascontiguousarray(
            x[b].T.reshape(4, P, L).transpose(1, 0, 2)).astype(np.float16)
        xts.append(xt)

    # mx[p, dc, k*256 + s*128 + oo] = M_s[k, dc*128+p, oq*128+oo]
    mcat = np.stack([M_phi_plus, M_phi_minus], axis=1)  # [K, 2, D, O]
    mxs = []
    for oq in range(NOQ):
        m = mcat[:, :, :, oq * OS:(oq + 1) * OS]        # [K, 2, D, OS]
        m = m.transpose(2, 0, 1, 3).reshape(D, K * 2 * OS)  # [D, K*2*OS]
        mx = np.ascontiguousarray(
            m.reshape(4, P, K * 2 * OS).transpose(1, 0, 2)).astype(np.float16)
        mxs.append(mx)

    # ph[kh, q, pp, ((dq, kl, p))] = phi[d*P + p - pp, kh*KPH+kl], d = 4q+dq
    idx = np.arange(P)
    diff = idx[None, :] - idx[:, None]                  # [pp, p] = p - pp
    v = np.arange(NB)[:, None, None] * P + diff[None]   # [d, pp, p]
    valid = v >= 0
    phb = np.zeros((NB, P, P, K), dtype=np.float32)     # [d, pp, p, k]
    phb[valid] = phi[v[valid], :]
    # [d, pp, p, (kh, kl)] -> [kh, q, pp, dq, kl, p]
    phb = phb.reshape(4, 4, P, P, KH, KPH).transpose(4, 0, 2, 1, 5, 3)
    ph = np.ascontiguousarray(phb.reshape(KH, 4, P, 4 * KPH * P)).astype(np.float16)

    sg = np.ascontiguousarray(sgn[:P].reshape(P, 1))

    in_maps = []
    for b in range(B):
        for oq in range(NOQ):
            in_maps.append({"xt": xts[b], "mx": mxs[oq], "ph": ph, "sg": sg})
    return in_maps


def kernel(x, phi, M_phi_plus, M_phi_minus):
    from concourse.bass_utils import run_bass_kernel_spmd

    x = np.asarray(x, dtype=np.float32)
    phi = np.asarray(phi, dtype=np.float32)
    M_phi_plus = np.asarray(M_phi_plus, dtype=np.float32)
    M_phi_minus = np.asarray(M_phi_minus, dtype=np.float32)

    if "nc" not in _cache:
        _cache["nc"] = _build_bass()
    nc = _cache["nc"]

    in_maps = _prep_inputs(x, phi, M_phi_plus, M_phi_minus)
    results = run_bass_kernel_spmd(nc, in_maps, core_ids=list(range(N_CORES)))

    out = np.empty((B, L, O), dtype=np.float32)
    for c in range(N_CORES):
        b, oq = divmod(c, NOQ)
        r = results.results[c]["out"]                   # [P, NB*OS]
        blk = r.reshape(P, NB, OS).transpose(1, 0, 2).reshape(L, OS)
        out[b, :, oq * OS:(oq + 1) * OS] = blk
    return out
